# revision 22
# baseline (speedup 1.0000x reference)
# Multi-head attention (B=4, T=2048, C=1024, H=16, D=64) on 8 trn2 NeuronCores.
#
# Sharding: 64 (batch, head) pairs -> 8 per core. Core c handles batch c//2,
# heads 8*(c%2) .. 8*(c%2)+8, i.e. a contiguous [2048, 512] column slice of x
# (and of the output).
#
# The tiny projections (Q/K/V = x @ W.T + b, 3% of FLOPs) are done host-side
# with BLAS and shipped in device-ready bf16 layouts; the 1/sqrt(64) score
# scale is folded into Q, the output bias bv and the softmax-denominator ones
# column are baked into V. The device runs pure flash attention per head pair
# A,B (= one 128-channel block), as ONE flat software-pipelined stream over
# (pair, ch, si) so the PE never drains at chunk/pair boundaries:
#   per iteration v = (pair, ch in 4 query chunks of 512, si in 16 key tiles):
#     sAB [128 keys, 512 qA | 512 qB] = KT_h.T @ QT_h  (one 2-bank PSUM tile;
#         the two 64-contraction matmuls run row-grouped concurrently)
#     pt = exp(sAB) -> bf16, alternating whole tiles between ScalarE (real
#         exp) and VectorE (Schraudolph int16 bit-trick) to split the load
#     oA[65,512] += [V_A|1].T @ pt[:,0:512]; oB likewise (row 64 accumulates
#         the softmax denominator for free)
#   S-batches of 3 key-tiles run ahead of the AV matmuls (PSUM ring depth 3);
#   oA/oB -> SBUF staging (Act/DVE), one DMA per pair; host divides by the
#   denominator row and transposes during unsharding.
import numpy as np

B, T, C = 4, 2048, 1024
H, D = 16, 64
NCORES = 8
PCOLS = C // 2          # 512 columns per core
TO = T // 128           # 16 key tiles
NPAIR = PCOLS // 128    # 4 head pairs per core
NCH = 4                 # 512-token query chunks
NIT = NPAIR * NCH * TO  # 256 flat iterations

_cached_nc = None


def _build_nc(reps=1):
    import concourse.bass as bass
    import concourse.mybir as mybir
    import concourse.tile as tile
    from concourse import bacc

    f32 = mybir.dt.float32
    bf16 = mybir.dt.bfloat16
    i16 = mybir.dt.int16
    AF = mybir.ActivationFunctionType
    ALU = mybir.AluOpType

    # Schraudolph exp constants for bf16 bit-construction via int16:
    # bits = round(s * 128/ln2 + (127*128 - 5)); scores arrive pre-scaled.
    SCHRA_A = float(np.float32((1 << 7) / np.log(2.0)))
    SCHRA_B = float(np.float32(127.0 * 128 - 5.0))
    nc = bacc.Bacc("TRN2", target_bir_lowering=False, debug=False)

    qt = nc.dram_tensor("qt", [PCOLS, T], bf16, kind="ExternalInput")
    kt = nc.dram_tensor("kt", [PCOLS, T], bf16, kind="ExternalInput")
    vh = nc.dram_tensor("vh", [NPAIR, 128, TO, 130], bf16,
                        kind="ExternalInput")
    # un-normalized O.T plus denominator row; [p][ch][65][qA|qB]
    yst = nc.dram_tensor("yst", [NPAIR, NCH, 65, 1024], f32,
                         kind="ExternalOutput")

    with tile.TileContext(nc) as tc:
        from contextlib import ExitStack

        with ExitStack() as ctx:
            qkp = ctx.enter_context(tc.tile_pool(name="qkp", bufs=2))
            vp = ctx.enter_context(tc.tile_pool(name="vp", bufs=2))
            ptp = ctx.enter_context(tc.tile_pool(name="ptp", bufs=4))
            osp = ctx.enter_context(tc.tile_pool(name="osp", bufs=3))
            # PSUM: s 3x[128,1024] (6 banks) + oA + oB = 8 banks
            ps_s = ctx.enter_context(tc.tile_pool(name="ps_s", bufs=3,
                                                  space="PSUM"))
            ps_o = ctx.enter_context(tc.tile_pool(name="ps_o", bufs=1,
                                                  space="PSUM"))

            import contextlib
            loop_cm = tc.For_i(0, reps, 1) if reps > 1 else \
                contextlib.nullcontext()
            with loop_cm:
                pair_tiles = {}
                od = {}
                pts = {}

                def ensure_pair(p):
                    if p in pair_tiles or p >= NPAIR:
                        return
                    KT2 = qkp.tile([128, T], bf16, tag="kt",
                                   name=f"kt{p}")
                    QT2 = qkp.tile([128, T], bf16, tag="qt",
                                   name=f"qt{p}")
                    for ch in range(NCH):
                        nc.sync.dma_start(
                            KT2[:, 512 * ch:512 * (ch + 1)],
                            kt[p * 128:(p + 1) * 128,
                               512 * ch:512 * (ch + 1)])
                        nc.sync.dma_start(
                            QT2[:, 512 * ch:512 * (ch + 1)],
                            qt[p * 128:(p + 1) * 128,
                               512 * ch:512 * (ch + 1)])
                    V2 = vp.tile([128, TO, 130], bf16, tag="v",
                                 name=f"v{p}")
                    nc.sync.dma_start(V2[:], vh[p])
                    pair_tiles[p] = (KT2, QT2, V2)

                def it(v):
                    p = v // (NCH * TO)
                    ch = (v // TO) % NCH
                    si = v % TO
                    return p, ch, si

                def s_exp(v):
                    p, ch, si = it(v)
                    KT2, QT2, V2 = pair_tiles[p]
                    sAB = ps_s.tile([128, 1024], f32, tag="s", name="sAB")
                    nc.tensor.matmul(sAB[:, 0:512],
                                     KT2[0:64, 128 * si:128 * (si + 1)],
                                     QT2[0:64, 512 * ch:512 * (ch + 1)],
                                     start=True, stop=True)
                    nc.tensor.matmul(sAB[:, 512:1024],
                                     KT2[64:128, 128 * si:128 * (si + 1)],
                                     QT2[64:128, 512 * ch:512 * (ch + 1)],
                                     start=True, stop=True)
                    # alternate exp between ScalarE and VectorE per tile
                    if si % 2 == 0:
                        pt = ptp.tile([128, 1024], bf16, tag="pta",
                                      name="pta")
                        nc.scalar.activation(pt, sAB, AF.Exp)
                        rhs = pt
                    else:
                        pt16 = ptp.tile([128, 1024], i16, tag="ptb",
                                        name="ptb")
                        nc.vector.tensor_scalar(
                            out=pt16[:], in0=sAB,
                            scalar1=SCHRA_A, scalar2=SCHRA_B,
                            op0=ALU.mult, op1=ALU.add)
                        rhs = pt16[:].bitcast(bf16)
                    pts[v] = rhs

                def av(v):
                    p, ch, si = it(v)
                    KT2, QT2, V2 = pair_tiles[p]
                    if si == 0:
                        oA = ps_o.tile([65, 512], f32, tag="oA", name="oA")
                        oB = ps_o.tile([65, 512], f32, tag="oB", name="oB")
                        od[(p, ch)] = (oA, oB)
                    oA, oB = od[(p, ch)]
                    rhs = pts.pop(v)
                    nc.tensor.matmul(oA, V2[:, si, 0:65], rhs[:, 0:512],
                                     start=(si == 0), stop=(si == TO - 1))
                    nc.tensor.matmul(oB, V2[:, si, 65:130], rhs[:, 512:1024],
                                     start=(si == 0), stop=(si == TO - 1))
                    if si == TO - 1:
                        # drain both halves on Act: its last exp here was
                        # si=14, so its queue is free while DVE still runs
                        # exp(15); frees oA/oB fastest for the next chunk
                        stc = osp.tile([65, 1024], f32, tag="st", name="st")
                        nc.scalar.activation(stc[:, 0:512], oA, AF.Identity)
                        nc.scalar.activation(stc[:, 512:1024], oB,
                                             AF.Identity)
                        nc.sync.dma_start(yst[p, ch], stc)
                        del od[(p, ch)]
                        if ch == NCH - 1:
                            del pair_tiles[p]

                ensure_pair(0)
                s_exp(0)
                s_exp(1)
                nxt = 2      # next iteration to emit S/exp for
                v = 0        # next iteration to emit AV for
                while v < NIT:
                    # emit S for up to 3 tiles ahead, then drain their AVs
                    batch = min(3, NIT - nxt)
                    for _ in range(batch):
                        p_nxt, ch_nxt, si_nxt = it(nxt)
                        if si_nxt == 0 and ch_nxt == 3:
                            ensure_pair(p_nxt + 1)  # prefetch next pair DMA
                        s_exp(nxt)
                        nxt += 1
                    for _ in range(max(batch, 1) if nxt < NIT else NIT - v):
                        av(v)
                        v += 1
                pair_tiles.clear()
                od.clear()
                pts.clear()
    nc.compile()
    return nc


def _host_inputs(x, Wq, bq, Wk, bk, Wv, bv):
    import ml_dtypes

    bf16 = ml_dtypes.bfloat16
    s = np.float32(0.125)  # 1/sqrt(64), folded into Q

    x2 = np.ascontiguousarray(x, dtype=np.float32).reshape(B * T, H, D)
    q = np.einsum("thd,ed->the", x2, Wq * s, optimize=True) + bq * s
    k = np.einsum("thd,ed->the", x2, Wk, optimize=True) + bk
    v = np.einsum("thd,ed->the", x2, Wv, optimize=True) + bv
    q = q.reshape(B, T, C)
    k = k.reshape(B, T, C)
    v = v.reshape(B, T, C)

    in_maps = []
    for c in range(NCORES):
        b, half = c // 2, c % 2
        sl = slice(half * PCOLS, (half + 1) * PCOLS)
        qtc = np.ascontiguousarray(q[b, :, sl].T).astype(bf16)
        ktc = np.ascontiguousarray(k[b, :, sl].T).astype(bf16)
        vc = v[b, :, sl]  # [T, 512]
        vhc = np.ones((NPAIR, TO, 128, 130), dtype=np.float32)
        vr = vc.reshape(TO, 128, NPAIR, 2, 64)
        for p in range(NPAIR):
            vhc[p, :, :, 0:64] = vr[:, :, p, 0]
            vhc[p, :, :, 65:129] = vr[:, :, p, 1]
        # -> [pair, 128 part, TO, 130] so the DMA is contiguous per partition
        vhc = np.ascontiguousarray(vhc.transpose(0, 2, 1, 3))
        in_maps.append({
            "qt": qtc, "kt": ktc, "vh": vhc.astype(bf16),
        })
    return in_maps


def _assemble(results, bv):
    y = np.empty((B, T, C), dtype=np.float32)
    for c in range(NCORES):
        b, half = c // 2, c % 2
        blk = results[c]["yst"]          # [pair, ch, 65, 1024 (qA|qB)]
        blk = blk.reshape(NPAIR, NCH, 65, 2, 512)
        vals = blk[:, :, 0:64]           # [pair, ch, 64 e, head, 512 t]
        den = blk[:, :, 64:65]
        out = vals / den                 # normalized (bv baked into V)
        # [p, ch, e, h, t] -> [ch, t, p, h, e] -> [2048 t, 512 c]
        out = np.ascontiguousarray(out.transpose(1, 4, 0, 3, 2))
        y[b, :, half * PCOLS:(half + 1) * PCOLS] = out.reshape(T, PCOLS)
    return y


def _run(x, Wq, bq, Wk, bk, Wv, bv, trace=False):
    from concourse.bass_utils import run_bass_kernel_spmd

    global _cached_nc
    if _cached_nc is None:
        _cached_nc = _build_nc()
    in_maps = _host_inputs(x, Wq, bq, Wk, bk, Wv, bv)
    res = run_bass_kernel_spmd(_cached_nc, in_maps,
                               core_ids=list(range(NCORES)), trace=trace)
    y = _assemble(res.results, np.asarray(bv))
    return y, res


def kernel(x, Wq, bq, Wk, bk, Wv, bv):
    y, _ = _run(np.asarray(x), np.asarray(Wq), np.asarray(bq), np.asarray(Wk),
                np.asarray(bk), np.asarray(Wv), np.asarray(bv))
    return y
